# revision 16
# baseline (speedup 1.0000x reference)
"""Trainium2 Bass kernel for nn_Max_loss (sparse-signal window loss).

Reference semantics (FURTHEST=1, SIG_WEIGHT=30, CLOSE_MIN=0.05):
  src[y]   = O[y] if (O[y] != 0 and y >= 1) else 0
  om[t]    = src[t+1] if != 0 else (src[t] if != 0 else (src[t-1] if != 0 else O[t]))
  l1       = (R - O)^2
  l2       = (R - om)^3 + 0.05
  l        = min(l1, l2) * (30 if O != 0 else 1)
  out      = mean(l)

Sharding: pure data parallel over batch (64 images -> 8 cores x 8 images).
Each core computes partial sums (via per-instruction accum_out); the host
adds the 8x128 partials and divides.

All heavy math runs as five fused custom DVE ops per [128, 4*512] tile
(this toolchain rejects Pool-engine elementwise; fp32 PE identity-matmul
tricks are slower than DVE):
  SELPRI  x2 : om priority-select over shifted APs of the same tile
  CUBEP      : q2 = (R - om)^3 + 0.05
  SQDIFFM    : l1s = (R - O)^2 * (1 - 2*(O != 0))  (sign carries the mask)
  MINW       : sum += min(|l1s|, q2) * (1 + 29*(l1s < 0))  (8 ALU stages,
               fused reduction via accum_out)
"""

import numpy as np

import concourse.bacc as bacc
import concourse.mybir as mybir
from concourse.tile import TileContext
from concourse.bass_utils import run_bass_kernel_spmd
from concourse.dve_ops import DveOp, OPS, CUSTOM_DVE_SPECS, _SUB_OPCODE_FOR_NAME
from concourse.dve_spec import (
    Spec,
    Src0,
    Src1,
    C0,
    Zero,
    One,
    select,
    ne,
    sq,
    maxx,
    minn,
    lower,
    AluOp,
)
from concourse.dve_uop import DveOpSpec

F32 = mybir.dt.float32
ALU = mybir.AluOpType
ACTF = mybir.ActivationFunctionType

N_CORES = 8
B, C, H, W = 64, 1, 512, 512
B_PER = B // N_CORES          # 8 images per core
IMG_GROUP = 4                 # images packed side-by-side in a tile free dim
ROW_BLK = 128                 # partition dim = rows of the image
N_GI = B_PER // IMG_GROUP     # 2 image groups per core
N_RB = H // ROW_BLK           # 4 row blocks per image
CLOSE_MIN = 0.05
SIG_WEIGHT = 30.0


def _register(name, spec_body, reference, accum=None):
    if name in _SUB_OPCODE_FOR_NAME:  # already registered in this process
        return next(op for op in OPS if op.name == name)
    kw = {"accum": accum} if accum is not None else {}
    spec = Spec(body=spec_body, reference=reference, **kw)
    row = max(_SUB_OPCODE_FOR_NAME.values()) + 1
    shas = {}
    for ver in ("v3", "v4"):
        s = DveOpSpec(name=name, opcode=row, uops=lower(spec, ver=ver), rd1_en=True)
        shas[ver] = s.sha(ver)
    op = DveOp(name, spec, subdim=False, uops_sha=shas)
    OPS.append(op)
    CUSTOM_DVE_SPECS[name] = spec
    _SUB_OPCODE_FOR_NAME[name] = row
    return op


# out = in1 if in1 != 0 else in0   (priority overwrite, exact select)
SELPRI = _register(
    "SELPRI_ANT",
    select(ne(Src1, Zero), Src1, Src0),
    lambda in0, in1, s0, s1, imm2: np.where(in1 != 0, in1, in0).astype(np.float32),
)

# out = (in0 - in1)^3 + s0
_d = Src0 - Src1
CUBEP = _register(
    "CUBEP_ANT",
    sq(_d) * _d + C0,
    lambda in0, in1, s0, s1, imm2: ((in0 - in1) ** 3 + s0).astype(np.float32),
)

# out = (in0 - in1)^2 * (1 - 2*(in1 != 0))   (sign encodes the weight mask)
_b = ne(Src1, Zero)
SQDIFFM = _register(
    "SQDIFFM_ANT",
    sq(Src0 - Src1) * (One - (_b + _b)),
    lambda in0, in1, s0, s1, imm2: (
        (in0 - in1) ** 2 * (1.0 - 2.0 * (in1 != 0))
    ).astype(np.float32),
)

# in0 = sign-encoded l1, in1 = q2:
# out = min(|in0|, in1) * (1 + s0*(in0 < 0)) ; accum_out = sum(out)
_absl1 = maxx(Src0, Zero - Src0)
MINW = _register(
    "MINW_ANT",
    minn(_absl1, Src1) * ((Src0 < Zero) * C0 + One),
    lambda in0, in1, s0, s1, imm2: (
        np.minimum(np.abs(in0), in1) * (1.0 + s0 * (in0 < 0))
    ).astype(np.float32),
    accum=AluOp.ADD,
)


def _build_kernel():
    nc = bacc.Bacc(
        "TRN2", target_bir_lowering=False, debug=False, num_devices=N_CORES
    )
    r = nc.declare_dram_parameter("r", [B_PER, H, W], F32, isOutput=False)
    o = nc.declare_dram_parameter("o", [B_PER, H, W], F32, isOutput=False)
    out = nc.dram_tensor("out", [128, 1], F32, kind="ExternalOutput")

    # [B_PER, H, W] -> [gi, rb, p, j, w]: tile partition p = row-in-block,
    # free dims (j = image-in-group, w = y)
    r_v = r[:].rearrange("(gi j) (rb p) w -> gi rb p j w", j=IMG_GROUP, p=ROW_BLK)
    o_v = o[:].rearrange("(gi j) (rb p) w -> gi rb p j w", j=IMG_GROUP, p=ROW_BLK)

    n_tiles = N_GI * N_RB  # 8

    with TileContext(nc) as tc:
        with (
            tc.tile_pool(name="acc", bufs=1) as acc_pool,
            tc.tile_pool(name="work", bufs=3) as pool,
        ):
            accA = acc_pool.tile([128, n_tiles], F32)  # sum(w * l) per tile

            for g in range(n_tiles):
                gi, rb = divmod(g, N_RB)
                rT = pool.tile([128, IMG_GROUP, W], F32, tag="rT")
                oT = pool.tile([128, IMG_GROUP, W], F32, tag="oT")
                nc.sync.dma_start(out=rT[:], in_=r_v[gi, rb])
                nc.sync.dma_start(out=oT[:], in_=o_v[gi, rb])

                # --- om: priority select over the +-1 window along w (DVE) ---
                # om1[t] = O[t] if O[t]!=0 else O[t-1]   (t >= 2; edges = O[t])
                om1 = pool.tile([128, IMG_GROUP, W], F32, tag="om1")
                nc.scalar.copy(om1[:, :, 0:2], oT[:, :, 0:2])  # tiny edge
                nc.vector._custom_dve(
                    SELPRI,
                    out=om1[:, :, 2:W],
                    in0=oT[:, :, 1 : W - 1],
                    in1=oT[:, :, 2:W],
                )
                # om[t] = O[t+1] if O[t+1]!=0 else om1[t]  (t < W-1; in place)
                nc.vector._custom_dve(
                    SELPRI,
                    out=om1[:, :, 0 : W - 1],
                    in0=om1[:, :, 0 : W - 1],
                    in1=oT[:, :, 1:W],
                )
                # --- q2 = (R - om)^3 + 0.05 (DVE, in place on om1) ---
                nc.vector._custom_dve(
                    CUBEP,
                    out=om1[:],
                    in0=rT[:],
                    in1=om1[:],
                    s0=CLOSE_MIN,
                )

                # --- l1s = (R - O)^2 * (1 - 2*(O != 0))  (DVE custom) ---
                l1 = pool.tile([128, IMG_GROUP, W], F32, tag="l1")
                nc.vector._custom_dve(
                    SQDIFFM,
                    out=l1[:],
                    in0=rT[:],
                    in1=oT[:],
                )

                # --- accA[:, g] = sum(min(|l1s|, q2) * (1 + 29*(l1s < 0))) ---
                nc.vector._custom_dve(
                    MINW,
                    out=om1[:],
                    in0=l1[:],
                    in1=om1[:],
                    s0=SIG_WEIGHT - 1.0,
                    accum_out=accA[:, g : g + 1],
                )

            # --- final: out[:, 0] = rowsum(accA) ---
            red = acc_pool.tile([128, 1], F32)
            nc.vector.tensor_reduce(
                red[:, 0:1], accA[:], mybir.AxisListType.X, ALU.add
            )
            nc.sync.dma_start(out=out[:], in_=red[:])
    nc.compile()
    return nc


_NC = None


def kernel(reconstruction: np.ndarray, original: np.ndarray) -> np.ndarray:
    global _NC
    if _NC is None:
        _NC = _build_kernel()

    r = np.ascontiguousarray(reconstruction.reshape(B, H, W), dtype=np.float32)
    o = np.ascontiguousarray(original.reshape(B, H, W), dtype=np.float32)

    in_maps = [
        {
            "r": r[c * B_PER : (c + 1) * B_PER],
            "o": o[c * B_PER : (c + 1) * B_PER],
        }
        for c in range(N_CORES)
    ]
    res = run_bass_kernel_spmd(_NC, in_maps, list(range(N_CORES))).results
    total = 0.0
    for c in range(N_CORES):
        outc = res[c]["out"].astype(np.float64)
        total += outc.sum()
    mean = total / (B * C * H * W)
    return np.float32(mean)
